# revision 5
# baseline (speedup 1.0000x reference)
"""Trainium2 Bass kernel v2 for nn_DiagonalVariance.

Key changes vs v1:
  - fp16 matmul datapath (x, weights, hidden) — PE still 1 cyc/row, half SBUF.
  - NB=1024 with a 3-buffer rotating PSUM pool for z1/z2 (6 banks) plus a
    dedicated z3 accumulator (2 banks): kills the v1 tile-end stall where all
    32 L3 matmuls ran serially against a PSUM bank shared with z1.
  - L3 accumulation interleaved per pair-group.
  - DVE offload: for `dve_pairs` of the 8 dim-pairs, L2's softplus moves off
    ScalarE onto the (previously idle) DVE as
        softplus(z) = z/2 + |z|/2 + R(min(|z|,8)),  R = (2,2) minimax rational
    The z/2 term is folded into the L3 matmul via a host-precomputed
    0.5*W2@W3 stationary (plus b2/alpha corrections folded into b3), so the
    DVE chain is 9 ops, only one of which reads PSUM.
"""

import numpy as np
from contextlib import ExitStack, nullcontext

import concourse.bass as bass
import concourse.bacc as bacc
import concourse.tile as tile
from concourse import mybir
from concourse.hw_specs import get_activation_tables

F = mybir.ActivationFunctionType
ALU = mybir.AluOpType
FP32 = mybir.dt.float32
FP16 = mybir.dt.float16

B = 262144
D = 16
TE = 3
H = 64
NCORES = 8
BC = B // NCORES          # 32768 batch points per core
NB = 1024                 # batch tile
NMM = 512                 # max moving free dim per matmul
NPAIR = D // 2            # 8 dim-pairs
NTILES = BC // NB         # 32

_ACT_SET = "natural_log_exp_and_others"

# (2,2) minimax rational for w(v)=ln(1+e^-v) on [0,8], err<=1.8e-3:
#   R(v) = (p0+p1 v+p2 v^2)/(1+q1 v+q2 v^2) = ALPHA + (BETA + GAMMA v)/den
_P = [0.691395, -0.20934, 0.015957]
_Q = [0.376745, 0.22133]
ALPHA = _P[2] / _Q[1]
BETA = _P[0] - ALPHA
GAMMA = _P[1] - ALPHA * _Q[0]
VCLAMP = 8.0


def softplus_dve_ref(z):
    """Numpy model of the DVE chain + matmul fold (for numerics checks)."""
    f16 = lambda x: x.astype(np.float16).astype(np.float64)
    v0 = f16(np.abs(z))
    v0h = f16(v0 * 0.5)
    vc = f16(np.minimum(v0, VCLAMP))
    t = f16(_Q[1] * vc + _Q[0])
    tv = f16(t * vc)
    den = f16(tv + 1.0)
    rec = f16(1.0 / den)
    L = f16(GAMMA * vc + BETA)
    wm = f16(L * rec)
    h2phi = f16(v0h + wm)
    return h2phi + 0.5 * z + ALPHA      # the latter two live in the matmul/bias


def _pin_act_tables(arch):
    tables = get_activation_tables(arch)
    for name, funcs in tables.items():
        if name != _ACT_SET:
            funcs.discard(F.Exp)
            funcs.discard(F.Ln)


def build(ntiles=NTILES, reps=1, nb=NB, dve_pairs=2, ln_group=2, l1_pairs=0, asserts=False):
    nc = bacc.Bacc("TRN2", target_bir_lowering=False, debug=False,
                   enable_asserts=asserts, num_devices=NCORES)
    _pin_act_tables(nc.m.arch)
    G = ln_group
    NGRP = NPAIR // G
    # offloaded pairs: the last pair of each group, up to dve_pairs
    offl = set()
    for g in range(NGRP):
        if len(offl) < dve_pairs:
            offl.add(g * G + (G - 1))
    for g in range(NGRP):
        for j in range(G - 1):
            if len(offl) < dve_pairs:
                offl.add(g * G + j)
    l1off = set(sorted(set(range(NPAIR)) - offl)[:l1_pairs])

    xT = nc.dram_tensor("xT", [20, BC], FP16, kind="ExternalInput")
    w1 = nc.dram_tensor("w1", [20, NPAIR * 128], FP16, kind="ExternalInput")
    w2 = nc.dram_tensor("w2", [128, NPAIR * 128], FP16, kind="ExternalInput")
    w3 = nc.dram_tensor("w3", [128, NPAIR * 128], FP16, kind="ExternalInput")
    wx = nc.dram_tensor("wx", [128, NPAIR * 128], FP16, kind="ExternalInput")
    wy = nc.dram_tensor("wy", [20, NPAIR * 128], FP16, kind="ExternalInput")
    b2 = nc.dram_tensor("b2", [128, NPAIR], FP32, kind="ExternalInput")
    b3 = nc.dram_tensor("b3", [128, 1], FP32, kind="ExternalInput")
    out = nc.dram_tensor("out", [D, ntiles * nb], FP32, kind="ExternalOutput")

    mm = nc.tensor.matmul

    with tile.TileContext(nc) as tc:
        with ExitStack() as ctx:
            wpool = ctx.enter_context(tc.tile_pool(name="w", bufs=1))
            xpool = ctx.enter_context(tc.tile_pool(name="x", bufs=2))
            e1pool = ctx.enter_context(tc.tile_pool(name="e1", bufs=2))
            h1pool = ctx.enter_context(tc.tile_pool(name="h1", bufs=2))
            e2pool = ctx.enter_context(tc.tile_pool(name="e2", bufs=2))
            h2pool = ctx.enter_context(tc.tile_pool(name="h2", bufs=2))
            vpool = ctx.enter_context(tc.tile_pool(name="v", bufs=2))
            tpool = ctx.enter_context(tc.tile_pool(name="t", bufs=2))
            gpool = ctx.enter_context(tc.tile_pool(name="g", bufs=2))
            e3pool = ctx.enter_context(tc.tile_pool(name="e3", bufs=2))
            opool = ctx.enter_context(tc.tile_pool(name="o", bufs=2))
            zpool = ctx.enter_context(tc.tile_pool(name="z", bufs=2, space="PSUM"))
            z3pool = ctx.enter_context(tc.tile_pool(name="z3", bufs=2, space="PSUM"))

            w1sb = wpool.tile([20, NPAIR * 128], FP16)
            w2sb = wpool.tile([128, NPAIR * 128], FP16)
            w3sb = wpool.tile([128, NPAIR * 128], FP16)
            wxsb = wpool.tile([128, NPAIR * 128], FP16)
            wysb = wpool.tile([20, NPAIR * 128], FP16)
            b2sb = wpool.tile([128, NPAIR], FP32)
            b3sb = wpool.tile([128, 1], FP32)
            nc.sync.dma_start(out=w1sb, in_=w1[:, :])
            nc.sync.dma_start(out=w2sb, in_=w2[:, :])
            nc.sync.dma_start(out=w3sb, in_=w3[:, :])
            nc.sync.dma_start(out=wxsb, in_=wx[:, :])
            nc.sync.dma_start(out=wysb, in_=wy[:, :])
            nc.sync.dma_start(out=b2sb, in_=b2[:, :])
            nc.sync.dma_start(out=b3sb, in_=b3[:, :])

            nchunk = nb // NMM

            def mm_chunks(zt, lhsT, rhs):
                for m in range(nchunk):
                    s = slice(m * NMM, (m + 1) * NMM)
                    mm(zt[:, s], lhsT, rhs[:, s], start=True, stop=True)

            def dve_phi(zin):
                """phi(z) = |z|/2 + R(min(|z|,8)) on DVE, fp16."""
                with nc.allow_low_precision(reason="fp16 softplus tail"):
                    nz = tpool.tile([128, nb], FP16, tag="nz")
                    nc.vector.tensor_scalar(
                        nz, zin, scalar1=-1.0, op0=ALU.mult,
                        scalar2=1.0, op1=ALU.mult)
                    v0 = vpool.tile([128, nb], FP16, tag="v0")
                    nc.vector.tensor_tensor(v0, zin, nz, op=ALU.max)
                    v0h = vpool.tile([128, nb], FP16)
                    nc.vector.tensor_scalar(
                        v0h, v0, scalar1=0.5, op0=ALU.mult,
                        scalar2=1.0, op1=ALU.mult)
                    vc = tpool.tile([128, nb], FP16)
                    nc.vector.tensor_scalar(
                        vc, v0, scalar1=VCLAMP, op0=ALU.min,
                        scalar2=1.0, op1=ALU.mult)
                    t1 = tpool.tile([128, nb], FP16)
                    nc.vector.tensor_scalar(
                        t1, vc, scalar1=_Q[1], op0=ALU.mult,
                        scalar2=_Q[0], op1=ALU.add)
                    tv = tpool.tile([128, nb], FP16)
                    nc.vector.tensor_tensor(tv, t1, vc, op=ALU.mult)
                    den = tpool.tile([128, nb], FP16)
                    nc.vector.tensor_scalar(
                        den, tv, scalar1=1.0, op0=ALU.add,
                        scalar2=1.0, op1=ALU.mult)
                    rec = tpool.tile([128, nb], FP16)
                    nc.vector.reciprocal(rec, den)
                    lnum = tpool.tile([128, nb], FP16)
                    nc.vector.tensor_scalar(
                        lnum, vc, scalar1=GAMMA, op0=ALU.mult,
                        scalar2=BETA, op1=ALU.add)
                    wm = tpool.tile([128, nb], FP16)
                    nc.vector.tensor_tensor(wm, lnum, rec, op=ALU.mult)
                    hphi = gpool.tile([128, nb], FP16)
                    nc.vector.tensor_tensor(hphi, v0h, wm, op=ALU.add)
                return hphi

            loop_cm = tc.For_i(0, reps, 1) if reps > 1 else nullcontext()
            with loop_cm:
                pending = []        # deferred output-layer work: (z3, i)

                def flush_pending():
                    while pending:
                        z3p, ip = pending.pop(0)
                        e3 = e3pool.tile([16, nb], FP16)
                        nc.scalar.activation(e3, z3p[:D, :], F.Exp,
                                             bias=b3sb[:D, :])
                        o3 = opool.tile([D, nb], FP32)
                        nc.scalar.activation(o3, e3, F.Ln, bias=1.0)
                        nc.sync.dma_start(out=out[:, ip * nb:(ip + 1) * nb],
                                          in_=o3)

                for i in range(ntiles):
                    xt = xpool.tile([20, nb], FP16)
                    nc.sync.dma_start(out=xt, in_=xT[:, i * nb:(i + 1) * nb])

                    z3 = z3pool.tile([128, nb], FP32)
                    z3_first = [True] * nchunk
                    n_l3 = [NPAIR + dve_pairs] * nchunk  # mms expected per chunk

                    def l3_mm(lhsT, rhs, m):
                        s = slice(m * NMM, (m + 1) * NMM)
                        st = z3_first[m]
                        z3_first[m] = False
                        n_l3[m] -= 1
                        mm(z3[:, s], lhsT, rhs[:, s], start=st, stop=(n_l3[m] == 0))

                    for g in range(NGRP):
                        pg = list(range(g * G, (g + 1) * G))
                        # ACT-L2 pairs first so their Ln1 lands earliest
                        pg.sort(key=lambda p: p in offl)
                        # ---- L1: per-pair Exp + per-pair Ln ----
                        h1s = {}
                        for p in pg:
                            z1 = zpool.tile([128, nb], FP32, tag="z")
                            mm_chunks(z1, w1sb[:, p * 128:(p + 1) * 128], xt)
                            if p in l1off:
                                h1s[p] = dve_phi(z1)   # z1/2 folded via wy mm
                            else:
                                e1 = e1pool.tile([128, nb], FP16, tag="e1")
                                nc.scalar.activation(e1, z1, F.Exp)
                                h1 = h1pool.tile([128, nb], FP16, tag="h1")
                                nc.scalar.activation(h1, e1, F.Ln, bias=1.0)
                                h1s[p] = h1
                        if g == 0 and pending:
                            flush_pending()

                        # ---- L2 + L3 per pair ----
                        for p in pg:
                            z2 = zpool.tile([128, nb], FP32, tag="z")
                            if p in l1off:
                                for m in range(nchunk):
                                    s = slice(m * NMM, (m + 1) * NMM)
                                    mm(z2[:, s], w2sb[:, p * 128:(p + 1) * 128],
                                       h1s[p][:, s], start=True, stop=False)
                                    mm(z2[:, s], wysb[:, p * 128:(p + 1) * 128],
                                       xt[:, s], start=False, stop=True)
                            else:
                                mm_chunks(z2, w2sb[:, p * 128:(p + 1) * 128],
                                          h1s[p])
                            if p in offl:
                                h2f = dve_phi(z2)
                                for m in range(nchunk):
                                    l3_mm(w3sb[:, p * 128:(p + 1) * 128], h2f, m)
                                for m in range(nchunk):
                                    l3_mm(wxsb[:, p * 128:(p + 1) * 128],
                                          h1s[p], m)
                            else:
                                e2 = e2pool.tile([128, nb], FP16)
                                nc.scalar.activation(e2, z2, F.Exp,
                                                     bias=b2sb[:, p:p + 1])
                                h2 = h2pool.tile([128, nb], FP16)
                                nc.scalar.activation(h2, e2, F.Ln, bias=1.0)
                                for m in range(nchunk):
                                    l3_mm(w3sb[:, p * 128:(p + 1) * 128], h2, m)

                    pending.append((z3, i))
                flush_pending()
    nc.compile()
    return nc


def _offloaded_pairs(dve_pairs=4, ln_group=2, l1_pairs=0):
    G = ln_group
    NGRP = NPAIR // G
    offl = []
    for g in range(NGRP):
        if len(offl) < dve_pairs:
            offl.append(g * G + (G - 1))
    for g in range(NGRP):
        for j in range(G - 1):
            if len(offl) < dve_pairs:
                offl.append(g * G + j)
    l1off = set(sorted(set(range(NPAIR)) - set(offl))[:l1_pairs])
    return set(offl), l1off


def _pack_inputs(t, y, W1, b1, W2, b2, W3, b3, dve_pairs=2, ln_group=2, l1_pairs=0):
    t = np.asarray(t, np.float32)
    y = np.asarray(y, np.float32)
    W1 = np.asarray(W1, np.float32)
    b1 = np.asarray(b1, np.float32)
    W2 = np.asarray(W2, np.float32)
    b2 = np.asarray(b2, np.float32)
    W3 = np.asarray(W3, np.float32)
    b3 = np.asarray(b3, np.float32)
    offl, l1off = _offloaded_pairs(dve_pairs, ln_group, l1_pairs)

    xT = np.empty((20, B), np.float16)
    xT[:D] = y.T
    xT[D:D + TE] = t.T
    xT[D + TE] = 1.0

    w1p = np.zeros((20, NPAIR * 128), np.float32)
    w2p = np.zeros((128, NPAIR * 128), np.float32)
    w3p = np.zeros((128, NPAIR * 128), np.float32)
    wxp = np.zeros((128, NPAIR * 128), np.float32)
    wyp = np.zeros((20, NPAIR * 128), np.float32)
    b2p = np.zeros((128, NPAIR), np.float32)
    b3p = np.zeros((128, 1), np.float32)
    for p in range(NPAIR):
        for a in range(2):
            d = 2 * p + a
            c = slice(p * 128 + 64 * a, p * 128 + 64 * a + 64)
            w1p[d, c] = W1[d, 0, :]
            w1p[D:D + TE, c] = W1[d, 1:1 + TE, :]
            w1p[D + TE, c] = b1[d, :]
            w2p[64 * a:64 * a + 64, p * 128 + 64 * a:p * 128 + 64 * a + 64] = W2[d]
            w3p[64 * a:64 * a + 64, p * 128 + d] = W3[d, :, 0]
            b2p[64 * a:64 * a + 64, p] = b2[d]
            b3p[d, 0] = b3[d, 0]
            if p in offl:
                # 0.5 * W2[d] @ W3[d] stationary for the z/2 linear fold
                wxp[64 * a:64 * a + 64, p * 128 + d] = \
                    0.5 * (W2[d] @ W3[d, :, 0])
                # fold 0.5*W3^T b2 and ALPHA*sum(W3) into b3
                b3p[d, 0] += 0.5 * float(W3[d, :, 0] @ b2[d]) \
                    + ALPHA * float(W3[d, :, 0].sum())
    for p in sorted(l1off):
        blk = slice(p * 128, (p + 1) * 128)
        # z2 fold: 0.5 * W1^T-packed @ W2-packed (contraction over h1)
        wyp[:, blk] = 0.5 * (w1p[:, blk] @ w2p[blk, blk])
        # ALPHA constant of h1 phi goes through W2 into b2
        b2p[:, p] += ALPHA * w2p[blk, blk].sum(axis=0)

    in_maps = []
    for c in range(NCORES):
        in_maps.append({
            "xT": np.ascontiguousarray(xT[:, c * BC:(c + 1) * BC]),
            "w1": w1p.astype(np.float16), "w2": w2p.astype(np.float16),
            "w3": w3p.astype(np.float16), "wx": wxp.astype(np.float16),
            "wy": wyp.astype(np.float16), "b2": b2p, "b3": b3p,
        })
    return in_maps


def _unpack_output(results):
    return np.concatenate([results[c]["out"].T for c in range(NCORES)], axis=0)


def make_runner(nc):
    """Build a reusable jitted SPMD callable for `nc` (axon PJRT path)."""
    import jax
    from jax.sharding import Mesh, PartitionSpec, NamedSharding
    from jax.experimental.shard_map import shard_map
    from concourse import bass2jax

    bass2jax.install_neuronx_cc_hook()
    partition_name = nc.partition_id_tensor.name if nc.partition_id_tensor else None
    in_names, out_names, out_avals = [], [], []
    for alloc in nc.m.functions[0].allocations:
        if not isinstance(alloc, mybir.MemoryLocationSet):
            continue
        name = alloc.memorylocations[0].name
        if alloc.kind == "ExternalInput":
            if name != partition_name:
                in_names.append(name)
        elif alloc.kind == "ExternalOutput":
            out_names.append(name)
            out_avals.append(jax.core.ShapedArray(tuple(alloc.tensor_shape),
                                                  mybir.dt.np(alloc.dtype)))
    all_in = in_names + out_names + ([partition_name] if partition_name else [])

    def _body(*args):
        operands = list(args)
        if partition_name is not None:
            operands.append(bass2jax.partition_id_tensor())
        outs = bass2jax._bass_exec_p.bind(
            *operands, out_avals=tuple(out_avals),
            in_names=tuple(all_in), out_names=tuple(out_names),
            lowering_input_output_aliases=(), sim_require_finite=True,
            sim_require_nnan=True, nc=nc)
        return tuple(outs)

    mesh = Mesh(np.asarray(jax.devices()[:NCORES]), ("core",))
    n = len(in_names) + len(out_names)
    sharded = jax.jit(shard_map(_body, mesh=mesh,
                                in_specs=(PartitionSpec("core"),) * n,
                                out_specs=(PartitionSpec("core"),) * len(out_names),
                                check_rep=False), keep_unused=True)
    shard0 = NamedSharding(mesh, PartitionSpec("core"))
    zeros = [jax.device_put(np.zeros((NCORES * a.shape[0], *a.shape[1:]), a.dtype),
                            shard0) for a in out_avals]

    def stage(in_maps):
        return [jax.device_put(
            np.concatenate([np.asarray(in_maps[c][nm]) for c in range(NCORES)], axis=0),
            shard0) for nm in in_names]

    def run_staged(dev_in):
        out_arrs = sharded(*dev_in, *zeros)
        jax.block_until_ready(out_arrs)
        return out_arrs

    def run(in_maps):
        out_arrs = run_staged(stage(in_maps))
        return [
            {name: np.asarray(out_arrs[i]).reshape(NCORES, *out_avals[i].shape)[c]
             for i, name in enumerate(out_names)}
            for c in range(NCORES)
        ]

    run.stage = stage
    run.run_staged = run_staged
    run.out_names = out_names
    run.out_avals = out_avals
    return run


_CACHED = {}


def _get_runner():
    if "runner" not in _CACHED:
        _CACHED["runner"] = make_runner(build())
    return _CACHED["runner"]


def kernel(t, y, W1, b1, W2, b2, W3, b3):
    run = _get_runner()
    in_maps = _pack_inputs(t, y, W1, b1, W2, b2, W3, b3)
    results = run(in_maps)
    return _unpack_output(results)

